# revision 26
# baseline (speedup 1.0000x reference)
"""Trainium2 Bass kernel for nn_BinaryDense: out = x @ (sum_k sign(b_k)*a_k) + bias.

Shapes (hardcoded): x [4096,4096] f32, b [4,4096,4096] f32, a [4,4096] f32,
bias [4096] f32 -> out [4096,4096] f32.

Strategy: tensor-parallel over the output (units) dim across 8 NeuronCores.
Core c owns O-columns [c*512, (c+1)*512).

Per core (measured ~260us HW; 8-core bf16 matmul roofline is ~219us):
  1. Build w[:, oc] = sum_k copysign(a[k,oc], b[k,:,oc]) on-chip (~3.1us per
     128-row tile on DVE). b arrives bf16 in [I, K, O_c] layout (k-major):
     copysign is two bitwise DVE ops ((b & 0x8000) | a, the AND done as
     packed int32 pairs at 2x_2P) and the k-sum is two fully-dense bf16
     adds (2x_1P): t = c[0:2] + c[2:4] (1024-wide), w = t[0] + t[1].
  2. One bf16 matmul x @ w with fp32 PSUM accumulation:
     lhsT = x^T tiles (host-pretransposed bf16), rhs = w tiles, K=4096.
     The contraction runs in k-blocks of 8 with an fp32 SBUF accumulator
     per m-tile: within a k-block the PE sweeps all 32 m-tiles (PSUM as
     4 tags x 2 bufs = 8 banks, double-buffered m-blocks of 4), so each
     w-tile is consumed at ~7us of PE work while the next k-block's
     w-tiles build on DVE -- the PE only chases the build during the
     first k-block, and per-w stalls stay under the ~3.4us HAM window so
     the PE clock stays at 2.4GHz. The next k-block's build chains are
     emitted interleaved between m-blocks (DVE program order alternates
     psum evicts and builds; emitting them separately serializes).
  3. acc init includes bias; final k-block evict adds acc+psum, store fp32.

Host side only reshapes/casts/shards (no math): x^T bf16, b -> [I,K,O] bf16,
a/bias broadcast rows.
"""

import sys

if "/opt/trn_rl_repo" not in sys.path:
    sys.path.insert(0, "/opt/trn_rl_repo")

import numpy as np
import ml_dtypes

BF16 = ml_dtypes.bfloat16

B = 4096   # batch rows of x
I = 4096   # input dim (contraction)
O = 4096   # output dim (sharded)
K = 4      # binary bases
NCORES = 8
OC = O // NCORES   # 512 output cols per core
P = 128

KT = I // P        # 32 k-tiles (contraction)
MT = B // P        # 32 m-tiles (output rows)
M_BLOCK = 4        # m-tiles per psum block (4 tags x 2 bufs = 8 banks)
K_BLOCK = 8        # k-tiles per accumulator pass


def _build_program():
    import concourse.bass as bass
    import concourse.mybir as mybir
    from concourse import bacc
    from concourse.tile import TileContext

    nc = bacc.Bacc(None, target_bir_lowering=False)

    b_re = nc.declare_dram_parameter("b_re", [I, K * OC], mybir.dt.bfloat16, isOutput=False)
    a_b = nc.declare_dram_parameter("a_b", [P, K * OC], mybir.dt.bfloat16, isOutput=False)
    xT = nc.declare_dram_parameter("xT", [I, B], mybir.dt.bfloat16, isOutput=False)
    bias_b = nc.declare_dram_parameter("bias_b", [P, OC], mybir.dt.float32, isOutput=False)
    out = nc.declare_dram_parameter("out", [B, OC], mybir.dt.float32, isOutput=True)

    with TileContext(nc) as tc:
        with (
            tc.tile_pool(name="const", bufs=1) as const,
            tc.tile_pool(name="bpool", bufs=8) as bpool,
            tc.tile_pool(name="cpool", bufs=4) as cpool,
            tc.tile_pool(name="tpool", bufs=4) as tpool,
            tc.tile_pool(name="wpool", bufs=1) as wpool,
            tc.tile_pool(name="xpool", bufs=12) as xpool,
            tc.tile_pool(name="opool", bufs=4) as opool,
            tc.tile_pool(name="apool", bufs=1) as apool,
            tc.tile_pool(name="psum", bufs=2, space="PSUM") as psum_pool,
        ):
            # consts on SWDGE so the HWDGE queue starts with b0/xt0
            a_tile = const.tile([P, K * OC], mybir.dt.bfloat16)
            nc.gpsimd.dma_start(out=a_tile[:], in_=a_b[:, :])
            bias_tile = const.tile([P, OC], mybir.dt.float32)
            nc.gpsimd.dma_start(out=bias_tile[:], in_=bias_b[:, :])
            mask_tile = const.tile([P, 1], mybir.dt.int32)
            nc.vector.memset(mask_tile[:], -2147450880)  # 0x80008000: bf16 sign pair

            # ---- phase 1: build w tiles [P, OC] bf16, one per k-tile ----
            # Software-pipelined emission: consecutive DVE instructions touch
            # different k-tiles so the per-instruction SBUF read-write bubble
            # overlaps independent work.
            import os
            ADD1_ON_GPSIMD = os.environ.get("BK_ADD1_GPSIMD", "0") == "1"
            ADD1_DMA = os.environ.get("BK_ADD1_DMA", "0") == "1"
            KB0_GPSIMD = os.environ.get("BK_KB0_GPSIMD", "0") == "1"

            b_tiles_live = {}
            contrib_live = {}
            t_live = {}
            w_tiles = [None] * KT

            def emit_dma(kt):
                b_tile = bpool.tile([P, K * OC], mybir.dt.bfloat16, name="b_tile")
                nc.sync.dma_start(out=b_tile[:], in_=b_re[kt * P:(kt + 1) * P, :])
                b_tiles_live[kt] = b_tile

            def emit_and(kt):
                b_tile = b_tiles_live[kt]
                # sign bits: b &= 0x8000 (in place, packed int32 pairs)
                nc.vector.tensor_scalar(
                    out=b_tile.bitcast(mybir.dt.int32)[:],
                    in0=b_tile.bitcast(mybir.dt.int32)[:],
                    scalar1=mask_tile[:, 0:1],
                    scalar2=None,
                    op0=mybir.AluOpType.bitwise_and,
                )

            def emit_or(kt):
                b_tile = b_tiles_live.pop(kt)
                contrib = cpool.tile([P, K * OC], mybir.dt.bfloat16, name="contrib")
                nc.vector.tensor_tensor(
                    out=contrib.bitcast(mybir.dt.int16)[:],
                    in0=b_tile.bitcast(mybir.dt.int16)[:],
                    in1=a_tile.bitcast(mybir.dt.int16)[:],
                    op=mybir.AluOpType.bitwise_or,
                )
                contrib_live[kt] = contrib

            def emit_add1(kt):
                contrib = contrib_live.pop(kt)
                t_tile = tpool.tile([P, 2 * OC], mybir.dt.bfloat16, name="t_tile")
                if KB0_GPSIMD and kt < K_BLOCK:
                    # during the first k-block GpSimd is idle (no out-DMAs
                    # yet); taking add1 off DVE shortens the startup chase
                    nc.gpsimd.tensor_tensor(
                        out=t_tile[:],
                        in0=contrib[:, 0:2 * OC],
                        in1=contrib[:, 2 * OC:4 * OC],
                        op=mybir.AluOpType.add,
                    )
                elif ADD1_DMA:
                    nc.gpsimd.dma_start(out=t_tile[:], in_=contrib[:, 0:2 * OC])
                    nc.gpsimd.dma_start(out=t_tile[:], in_=contrib[:, 2 * OC:4 * OC],
                                        accum_op=mybir.AluOpType.add)
                else:
                    eng = nc.gpsimd if ADD1_ON_GPSIMD else nc.vector
                    eng.tensor_tensor(
                        out=t_tile[:],
                        in0=contrib[:, 0:2 * OC],
                        in1=contrib[:, 2 * OC:4 * OC],
                        op=mybir.AluOpType.add,
                    )
                t_live[kt] = t_tile

            def emit_add2(kt):
                t_tile = t_live.pop(kt)
                w_tile = wpool.tile([P, OC], mybir.dt.bfloat16, name=f"w_{kt}")
                nc.vector.tensor_tensor(
                    out=w_tile[:],
                    in0=t_tile[:, 0:OC],
                    in1=t_tile[:, OC:2 * OC],
                    op=mybir.AluOpType.add,
                )
                w_tiles[kt] = w_tile

            def emit_build(kt):
                emit_dma(kt)
                emit_and(kt)
                emit_or(kt)
                emit_add1(kt)
                emit_add2(kt)

            # Critical path first: b0, then xt0, then the rest. The first MM
            # needs only w[0] (from b0) and xt0.
            xt_prefetch = []

            def prefetch_xt(kt):
                xt = xpool.tile([P, P * M_BLOCK], mybir.dt.bfloat16, name="xt")
                nc.sync.dma_start(out=xt[:], in_=xT[kt * P:(kt + 1) * P, 0:M_BLOCK * P])
                xt_prefetch.append(xt)

            K_BLOCKS = [int(s) for s in
                        os.environ.get("BK_KBLOCKS", "8,8,8,8").split(",")]
            assert sum(K_BLOCKS) == KT
            NKB = len(K_BLOCKS)
            NMB = MT // M_BLOCK
            k_starts = [sum(K_BLOCKS[:i]) for i in range(NKB)]

            emit_dma(0)
            prefetch_xt(0)
            emit_and(0)
            emit_or(0)
            emit_add1(0)
            emit_add2(0)
            for kt in range(1, K_BLOCKS[0]):
                emit_build(kt)
                prefetch_xt(kt)
            build_cursor = K_BLOCKS[0]

            # ---- phase 2: k-blocked matmul with fp32 SBUF accumulator ----
            # The next k-block's w-build is emitted interleaved between the
            # m-blocks so the DVE program order alternates evicts and builds.
            acc_tiles = {}
            for kb in range(NKB):
                k0 = k_starts[kb]
                KB = K_BLOCKS[kb]
                for mb in range(NMB):
                    # emit next k-block's builds evenly across this kb's m-blocks
                    if kb + 1 < NKB:
                        import math
                        target = k_starts[kb + 1] + math.ceil(
                            (mb + 1) * K_BLOCKS[kb + 1] / NMB)
                        while build_cursor < target:
                            emit_build(build_cursor)
                            build_cursor += 1
                    ms = [mb * M_BLOCK + j for j in range(M_BLOCK)]
                    ps_tiles = {
                        m: psum_pool.tile([P, OC], mybir.dt.float32, name=f"ps_{m % M_BLOCK}")
                        for m in ms
                    }
                    for kt in range(k0, k0 + KB):
                        if kb == 0 and mb == 0:
                            xt = xt_prefetch[kt]
                        else:
                            xt = xpool.tile([P, P * M_BLOCK], mybir.dt.bfloat16, name="xt")
                            nc.sync.dma_start(
                                out=xt[:],
                                in_=xT[kt * P:(kt + 1) * P,
                                      ms[0] * P:(ms[0] + M_BLOCK) * P],
                            )
                        for j, m in enumerate(ms):
                            nc.tensor.matmul(
                                ps_tiles[m][:],
                                xt[:, j * P:(j + 1) * P],
                                w_tiles[kt][:],
                                start=(kt == k0),
                                stop=(kt == k0 + KB - 1),
                            )
                    for m in ms:
                        if kb == 0:
                            acc = apool.tile([P, OC], mybir.dt.float32, name=f"acc_{m}")
                            nc.vector.tensor_tensor(
                                out=acc[:], in0=ps_tiles[m][:], in1=bias_tile[:],
                                op=mybir.AluOpType.add,
                            )
                            acc_tiles[m] = acc
                        elif kb < NKB - 1:
                            acc = acc_tiles[m]
                            nc.vector.tensor_tensor(
                                out=acc[:], in0=ps_tiles[m][:], in1=acc[:],
                                op=mybir.AluOpType.add,
                            )
                        else:
                            o_tile = opool.tile([P, OC], mybir.dt.float32, name="o_tile")
                            nc.vector.tensor_tensor(
                                out=o_tile[:], in0=ps_tiles[m][:], in1=acc_tiles[m][:],
                                op=mybir.AluOpType.add,
                            )
                            # last m-block: HWDGE queues are idle (x loads done);
                            # avoids the SWDGE drain on the kernel tail
                            st = nc.sync if mb == NMB - 1 else nc.gpsimd
                            st.dma_start(out=out[m * P:(m + 1) * P, :], in_=o_tile[:])

    nc.compile()
    return nc


_NC_CACHE = None


def _get_program():
    global _NC_CACHE
    if _NC_CACHE is None:
        _NC_CACHE = _build_program()
    return _NC_CACHE


def prep_inputs(x, b, a, bias):
    """Host-side shard/cast/layout only. Returns per-core input maps."""
    x = np.asarray(x, dtype=np.float32)
    b = np.asarray(b, dtype=np.float32)
    a = np.asarray(a, dtype=np.float32)
    bias = np.asarray(bias, dtype=np.float32)
    xT16 = np.ascontiguousarray(x.T).astype(BF16)          # [I, B] bf16
    b_iko = np.transpose(b, (1, 0, 2)).astype(BF16)        # [I, K, O] bf16
    bias32 = bias.astype(np.float32)
    a16 = a.astype(BF16)                                    # [K, O]

    in_maps = []
    for c in range(NCORES):
        sl = slice(c * OC, (c + 1) * OC)
        b_slice = np.ascontiguousarray(b_iko[:, :, sl]).reshape(I, K * OC)
        a_flat = np.ascontiguousarray(a16[:, sl]).reshape(1, K * OC)
        a_bcast = np.broadcast_to(a_flat, (P, K * OC)).copy()
        bias_bcast = np.broadcast_to(bias32[sl].reshape(1, OC), (P, OC)).copy()
        in_maps.append({
            "b_re": b_slice,
            "a_b": a_bcast,
            "xT": xT16,
            "bias_b": bias_bcast,
        })
    return in_maps


def run(in_maps, trace=False):
    from concourse.bass_utils import run_bass_kernel_spmd

    nc = _get_program()
    res = run_bass_kernel_spmd(nc, in_maps, list(range(NCORES)), trace=trace)
    return res


def kernel(x, b, a, bias):
    in_maps = prep_inputs(x, b, a, bias)
    res = run(in_maps)
    out = np.concatenate([res.results[c]["out"] for c in range(NCORES)], axis=1)
    return np.ascontiguousarray(out, dtype=np.float32)


if __name__ == "__main__":
    rng = np.random.default_rng(0)
    x = rng.standard_normal((B, I), dtype=np.float32)
    b = rng.standard_normal((K, I, O), dtype=np.float32)
    a = rng.random((K, O), dtype=np.float32)
    bias = rng.standard_normal(O, dtype=np.float32)
    out = kernel(x=x, b=b, a=a, bias=bias)
    w_eff = np.einsum('kio,ko->io', np.sign(b), a.astype(np.float64)).astype(np.float64)
    expected = x.astype(np.float64) @ w_eff + bias
    rel = np.linalg.norm(out - expected) / np.linalg.norm(expected)
    print(f"rel_err = {rel:.3e}")


# revision 27
# speedup vs baseline: 1.0214x; 1.0214x over previous
"""Trainium2 Bass kernel for nn_BinaryDense: out = x @ (sum_k sign(b_k)*a_k) + bias.

Shapes (hardcoded): x [4096,4096] f32, b [4,4096,4096] f32, a [4,4096] f32,
bias [4096] f32 -> out [4096,4096] f32.

Strategy: tensor-parallel over the output (units) dim across 8 NeuronCores.
Core c owns O-columns [c*512, (c+1)*512).

Per core (measured ~260us HW; 8-core bf16 matmul roofline is ~219us):
  1. Build w[:, oc] = sum_k copysign(a[k,oc], b[k,:,oc]) on-chip (~3.1us per
     128-row tile on DVE). b arrives bf16 in [I, K, O_c] layout (k-major):
     copysign is two bitwise DVE ops ((b & 0x8000) | a, the AND done as
     packed int32 pairs at 2x_2P) and the k-sum is two fully-dense bf16
     adds (2x_1P): t = c[0:2] + c[2:4] (1024-wide), w = t[0] + t[1].
  2. One bf16 matmul x @ w with fp32 PSUM accumulation:
     lhsT = x^T tiles (host-pretransposed bf16), rhs = w tiles, K=4096.
     The contraction runs in k-blocks of 8 with an fp32 SBUF accumulator
     per m-tile: within a k-block the PE sweeps all 32 m-tiles (PSUM as
     4 tags x 2 bufs = 8 banks, double-buffered m-blocks of 4), so each
     w-tile is consumed at ~7us of PE work while the next k-block's
     w-tiles build on DVE -- the PE only chases the build during the
     first k-block, and per-w stalls stay under the ~3.4us HAM window so
     the PE clock stays at 2.4GHz. The next k-block's build chains are
     emitted interleaved between m-blocks (DVE program order alternates
     psum evicts and builds; emitting them separately serializes).
  3. acc init includes bias; final k-block evict adds acc+psum, store fp32.

Host side only reshapes/casts/shards (no math): x^T bf16, b -> [I,K,O] bf16,
a/bias broadcast rows.
"""

import sys

if "/opt/trn_rl_repo" not in sys.path:
    sys.path.insert(0, "/opt/trn_rl_repo")

import numpy as np
import ml_dtypes

BF16 = ml_dtypes.bfloat16

B = 4096   # batch rows of x
I = 4096   # input dim (contraction)
O = 4096   # output dim (sharded)
K = 4      # binary bases
NCORES = 8
OC = O // NCORES   # 512 output cols per core
P = 128

KT = I // P        # 32 k-tiles (contraction)
MT = B // P        # 32 m-tiles (output rows)
M_BLOCK = 4        # m-tiles per psum block (4 tags x 2 bufs = 8 banks)
K_BLOCK = 8        # k-tiles per accumulator pass


def _build_program():
    import concourse.bass as bass
    import concourse.mybir as mybir
    from concourse import bacc
    from concourse.tile import TileContext

    nc = bacc.Bacc(None, target_bir_lowering=False)

    b_re = nc.declare_dram_parameter("b_re", [I, K * OC], mybir.dt.bfloat16, isOutput=False)
    a_b = nc.declare_dram_parameter("a_b", [P, K * OC], mybir.dt.bfloat16, isOutput=False)
    xT = nc.declare_dram_parameter("xT", [I, B], mybir.dt.bfloat16, isOutput=False)
    bias_b = nc.declare_dram_parameter("bias_b", [P, OC], mybir.dt.float32, isOutput=False)
    out = nc.declare_dram_parameter("out", [B, OC], mybir.dt.float32, isOutput=True)

    with TileContext(nc) as tc:
        with (
            tc.tile_pool(name="const", bufs=1) as const,
            tc.tile_pool(name="bpool", bufs=8) as bpool,
            tc.tile_pool(name="cpool", bufs=4) as cpool,
            tc.tile_pool(name="tpool", bufs=4) as tpool,
            tc.tile_pool(name="wpool", bufs=1) as wpool,
            tc.tile_pool(name="xpool", bufs=12) as xpool,
            tc.tile_pool(name="opool", bufs=4) as opool,
            tc.tile_pool(name="apool", bufs=1) as apool,
            tc.tile_pool(name="psum", bufs=2, space="PSUM") as psum_pool,
        ):
            # consts on SWDGE so the HWDGE queue starts with b0/xt0
            a_tile = const.tile([P, K * OC], mybir.dt.bfloat16)
            nc.gpsimd.dma_start(out=a_tile[:], in_=a_b[:, :])
            bias_tile = const.tile([P, OC], mybir.dt.float32)
            nc.gpsimd.dma_start(out=bias_tile[:], in_=bias_b[:, :])
            mask_tile = const.tile([P, 1], mybir.dt.int32)
            nc.vector.memset(mask_tile[:], -2147450880)  # 0x80008000: bf16 sign pair

            # ---- phase 1: build w tiles [P, OC] bf16, one per k-tile ----
            # Software-pipelined emission: consecutive DVE instructions touch
            # different k-tiles so the per-instruction SBUF read-write bubble
            # overlaps independent work.
            import os
            ADD1_ON_GPSIMD = os.environ.get("BK_ADD1_GPSIMD", "0") == "1"
            ADD1_DMA = os.environ.get("BK_ADD1_DMA", "0") == "1"
            KB0_GPSIMD = os.environ.get("BK_KB0_GPSIMD", "0") == "1"

            b_tiles_live = {}
            contrib_live = {}
            t_live = {}
            w_tiles = [None] * KT

            def emit_dma(kt):
                b_tile = bpool.tile([P, K * OC], mybir.dt.bfloat16, name="b_tile")
                nc.sync.dma_start(out=b_tile[:], in_=b_re[kt * P:(kt + 1) * P, :])
                b_tiles_live[kt] = b_tile

            def emit_and(kt):
                b_tile = b_tiles_live[kt]
                # sign bits: b &= 0x8000 (in place, packed int32 pairs)
                nc.vector.tensor_scalar(
                    out=b_tile.bitcast(mybir.dt.int32)[:],
                    in0=b_tile.bitcast(mybir.dt.int32)[:],
                    scalar1=mask_tile[:, 0:1],
                    scalar2=None,
                    op0=mybir.AluOpType.bitwise_and,
                )

            def emit_or(kt):
                b_tile = b_tiles_live.pop(kt)
                contrib = cpool.tile([P, K * OC], mybir.dt.bfloat16, name="contrib")
                nc.vector.tensor_tensor(
                    out=contrib.bitcast(mybir.dt.int16)[:],
                    in0=b_tile.bitcast(mybir.dt.int16)[:],
                    in1=a_tile.bitcast(mybir.dt.int16)[:],
                    op=mybir.AluOpType.bitwise_or,
                )
                contrib_live[kt] = contrib

            def emit_add1(kt):
                contrib = contrib_live.pop(kt)
                t_tile = tpool.tile([P, 2 * OC], mybir.dt.bfloat16, name="t_tile")
                if KB0_GPSIMD and kt < K_BLOCK:
                    # during the first k-block GpSimd is idle (no out-DMAs
                    # yet); taking add1 off DVE shortens the startup chase
                    nc.gpsimd.tensor_tensor(
                        out=t_tile[:],
                        in0=contrib[:, 0:2 * OC],
                        in1=contrib[:, 2 * OC:4 * OC],
                        op=mybir.AluOpType.add,
                    )
                elif ADD1_DMA:
                    nc.gpsimd.dma_start(out=t_tile[:], in_=contrib[:, 0:2 * OC])
                    nc.gpsimd.dma_start(out=t_tile[:], in_=contrib[:, 2 * OC:4 * OC],
                                        accum_op=mybir.AluOpType.add)
                else:
                    eng = nc.gpsimd if ADD1_ON_GPSIMD else nc.vector
                    eng.tensor_tensor(
                        out=t_tile[:],
                        in0=contrib[:, 0:2 * OC],
                        in1=contrib[:, 2 * OC:4 * OC],
                        op=mybir.AluOpType.add,
                    )
                t_live[kt] = t_tile

            def emit_add2(kt):
                t_tile = t_live.pop(kt)
                w_tile = wpool.tile([P, OC], mybir.dt.bfloat16, name=f"w_{kt}")
                nc.vector.tensor_tensor(
                    out=w_tile[:],
                    in0=t_tile[:, 0:OC],
                    in1=t_tile[:, OC:2 * OC],
                    op=mybir.AluOpType.add,
                )
                w_tiles[kt] = w_tile

            def emit_build(kt):
                emit_dma(kt)
                emit_and(kt)
                emit_or(kt)
                emit_add1(kt)
                emit_add2(kt)

            # Critical path first: b0, then xt0, then the rest. The first MM
            # needs only w[0] (from b0) and xt0.
            xt_prefetch = []

            def prefetch_xt(kt):
                xt = xpool.tile([P, P * M_BLOCK], mybir.dt.bfloat16, name="xt")
                nc.sync.dma_start(out=xt[:], in_=xT[kt * P:(kt + 1) * P, 0:M_BLOCK * P])
                xt_prefetch.append(xt)

            K_BLOCKS = [int(s) for s in
                        os.environ.get("BK_KBLOCKS", "8,8,8,8").split(",")]
            assert sum(K_BLOCKS) == KT
            NKB = len(K_BLOCKS)
            NMB = MT // M_BLOCK
            k_starts = [sum(K_BLOCKS[:i]) for i in range(NKB)]

            emit_dma(0)
            prefetch_xt(0)
            emit_and(0)
            emit_or(0)
            emit_add1(0)
            emit_add2(0)
            for kt in range(1, K_BLOCKS[0]):
                emit_build(kt)
                prefetch_xt(kt)
            build_cursor = K_BLOCKS[0]

            # ---- phase 2: k-blocked matmul with fp32 SBUF accumulator ----
            # The next k-block's w-build is emitted interleaved between the
            # m-blocks so the DVE program order alternates evicts and builds.
            acc_tiles = {}
            for kb in range(NKB):
                k0 = k_starts[kb]
                KB = K_BLOCKS[kb]
                for mb in range(NMB):
                    # emit next k-block's builds evenly across this kb's m-blocks
                    if kb + 1 < NKB:
                        import math
                        target = k_starts[kb + 1] + math.ceil(
                            (mb + 1) * K_BLOCKS[kb + 1] / NMB)
                        while build_cursor < target:
                            emit_build(build_cursor)
                            build_cursor += 1
                    ms = [mb * M_BLOCK + j for j in range(M_BLOCK)]
                    ps_tiles = {
                        m: psum_pool.tile([P, OC], mybir.dt.float32, name=f"ps_{m % M_BLOCK}")
                        for m in ms
                    }
                    for kt in range(k0, k0 + KB):
                        if kb == 0 and mb == 0:
                            xt = xt_prefetch[kt]
                        else:
                            xt = xpool.tile([P, P * M_BLOCK], mybir.dt.bfloat16, name="xt")
                            nc.sync.dma_start(
                                out=xt[:],
                                in_=xT[kt * P:(kt + 1) * P,
                                      ms[0] * P:(ms[0] + M_BLOCK) * P],
                            )
                        for j, m in enumerate(ms):
                            nc.tensor.matmul(
                                ps_tiles[m][:],
                                xt[:, j * P:(j + 1) * P],
                                w_tiles[kt][:],
                                start=(kt == k0),
                                stop=(kt == k0 + KB - 1),
                            )
                    for m in ms:
                        if kb == 0:
                            acc = apool.tile([P, OC], mybir.dt.float32, name=f"acc_{m}")
                            nc.vector.tensor_tensor(
                                out=acc[:], in0=ps_tiles[m][:], in1=bias_tile[:],
                                op=mybir.AluOpType.add,
                            )
                            acc_tiles[m] = acc
                        elif kb < NKB - 1:
                            acc = acc_tiles[m]
                            nc.vector.tensor_tensor(
                                out=acc[:], in0=ps_tiles[m][:], in1=acc[:],
                                op=mybir.AluOpType.add,
                            )
                        else:
                            o_tile = opool.tile([P, OC], mybir.dt.float32, name="o_tile")
                            nc.vector.tensor_tensor(
                                out=o_tile[:], in0=ps_tiles[m][:], in1=acc_tiles[m][:],
                                op=mybir.AluOpType.add,
                            )
                            nc.gpsimd.dma_start(out=out[m * P:(m + 1) * P, :], in_=o_tile[:])

    nc.compile()
    return nc


_NC_CACHE = None


def _get_program():
    global _NC_CACHE
    if _NC_CACHE is None:
        _NC_CACHE = _build_program()
    return _NC_CACHE


def prep_inputs(x, b, a, bias):
    """Host-side shard/cast/layout only. Returns per-core input maps."""
    x = np.asarray(x, dtype=np.float32)
    b = np.asarray(b, dtype=np.float32)
    a = np.asarray(a, dtype=np.float32)
    bias = np.asarray(bias, dtype=np.float32)
    xT16 = np.ascontiguousarray(x.T).astype(BF16)          # [I, B] bf16
    b_iko = np.transpose(b, (1, 0, 2)).astype(BF16)        # [I, K, O] bf16
    bias32 = bias.astype(np.float32)
    a16 = a.astype(BF16)                                    # [K, O]

    in_maps = []
    for c in range(NCORES):
        sl = slice(c * OC, (c + 1) * OC)
        b_slice = np.ascontiguousarray(b_iko[:, :, sl]).reshape(I, K * OC)
        a_flat = np.ascontiguousarray(a16[:, sl]).reshape(1, K * OC)
        a_bcast = np.broadcast_to(a_flat, (P, K * OC)).copy()
        bias_bcast = np.broadcast_to(bias32[sl].reshape(1, OC), (P, OC)).copy()
        in_maps.append({
            "b_re": b_slice,
            "a_b": a_bcast,
            "xT": xT16,
            "bias_b": bias_bcast,
        })
    return in_maps


def run(in_maps, trace=False):
    from concourse.bass_utils import run_bass_kernel_spmd

    nc = _get_program()
    res = run_bass_kernel_spmd(nc, in_maps, list(range(NCORES)), trace=trace)
    return res


def kernel(x, b, a, bias):
    in_maps = prep_inputs(x, b, a, bias)
    res = run(in_maps)
    out = np.concatenate([res.results[c]["out"] for c in range(NCORES)], axis=1)
    return np.ascontiguousarray(out, dtype=np.float32)


if __name__ == "__main__":
    rng = np.random.default_rng(0)
    x = rng.standard_normal((B, I), dtype=np.float32)
    b = rng.standard_normal((K, I, O), dtype=np.float32)
    a = rng.random((K, O), dtype=np.float32)
    bias = rng.standard_normal(O, dtype=np.float32)
    out = kernel(x=x, b=b, a=a, bias=bias)
    w_eff = np.einsum('kio,ko->io', np.sign(b), a.astype(np.float64)).astype(np.float64)
    expected = x.astype(np.float64) @ w_eff + bias
    rel = np.linalg.norm(out - expected) / np.linalg.norm(expected)
    print(f"rel_err = {rel:.3e}")
